# revision 6
# baseline (speedup 1.0000x reference)
"""BotGCN (nn_BotGCN_5901285065195) on 8 TRN2 NeuronCores.

Sharding: nodes split contiguously across the 8 cores (12544 padded rows per
core). Dense layers run H-major (features on partitions) so bias + LeakyReLU
fuse into single per-partition ACT ops. Each GCN layer does:
    table = AllGather(dinv * (x @ W))        # bf16 (fp32 for the H=64 layer)
    agg[d] = sum_{e: dst=d} table[src_e]     # dma_gather + selection matmuls
    x_next = dinv * agg + b
The per-edge gather uses dma_gather (int16 indices relative to one of 4
source-range buckets). The segment-sum is a matmul with a 0/1 selection
matrix built on DVE from per-edge dst values; pad edges carry dstrel=200 so
their selection column is all-zero and their gathered row is ignored.
"""
import sys

sys.path.insert(0, "/opt/trn_rl_repo")

import numpy as np
import concourse.bass as bass
import concourse.mybir as mybir
import concourse.bacc as bacc
import concourse.tile as tile
from concourse.bass_utils import run_bass_kernel_spmd

NC = 8
N = 100000
SH = 12500              # real nodes per core
SHP = 12544             # padded: 98 * 128
T = 98                  # dst tiles per core
NP = NC * SHP           # padded table rows (100352)
NBUCK = 4
BUCK = NP // NBUCK      # 25088 < 32768 (int16-safe)
SUP = 7                 # tiles per supertile; 98 = 14 * 7
NSUP = 14

f32 = mybir.dt.float32
bf16 = mybir.dt.bfloat16
i16 = mybir.dt.int16

WDIMS = {"des": (768, 32), "tweet": (768, 32), "num": (5, 32), "cat": (3, 32),
         "inp": (128, 128), "g1": (128, 128), "g2": (128, 128),
         "o1": (128, 64), "g3": (64, 64), "o2": (64, 64), "o3": (64, 32),
         "o4": (32, 16), "fin": (16, 2)}


def _prep(edge_index):
    """Host index prep -> (idx_dev, dr_dev, dinv_col, ktb)."""
    src = edge_index[0].astype(np.int64)
    dst = edge_index[1].astype(np.int64)
    loops = np.arange(N, dtype=np.int64)
    src = np.concatenate([src, loops])
    dst = np.concatenate([dst, loops])
    deg = np.bincount(dst, minlength=N).astype(np.float32)
    dinv = np.where(deg > 0, 1.0 / np.sqrt(deg), 0.0).astype(np.float32)

    gsrc = (src // SH) * SHP + (src % SH)      # padded table row of source
    core = dst // SH
    tile_id = (dst % SH) // 128
    drel = (dst % SH) % 128
    buck = gsrc // BUCK

    order = np.lexsort((buck, tile_id, core))
    gsrc, core, tile_id, drel, buck = (a[order] for a in
                                       (gsrc, core, tile_id, drel, buck))
    key = (core * T + tile_id) * NBUCK + buck
    cnt = np.bincount(key, minlength=NC * T * NBUCK)
    ktb = int(np.ceil(cnt.max() / 128))
    slots = ktb * 128

    idx16 = np.zeros((NC * T * NBUCK, slots), np.int16)
    drel_f = np.full((NC * T * NBUCK, slots), 200.0, np.float32)
    starts = np.zeros(NC * T * NBUCK + 1, np.int64)
    np.cumsum(cnt, out=starts[1:])
    grp = np.repeat(np.arange(NC * T * NBUCK), cnt)
    pos = np.arange(len(gsrc)) - starts[grp]
    idx16[grp, pos] = (gsrc - (grp % NBUCK) * BUCK).astype(np.int16)
    drel_f[grp, pos] = drel.astype(np.float32)
    idx16 = idx16.reshape(NC, T, NBUCK, slots)
    drel_f = drel_f.reshape(NC, T, NBUCK, slots)

    bsz = SUP * slots
    idx_dev = np.zeros((NC, NSUP, NBUCK, 128, bsz // 16), np.int16)
    dr_dev = np.zeros((NC, NSUP, NBUCK, 128, SUP * ktb), np.float32)
    for c in range(NC):
        for s in range(NSUP):
            for b in range(NBUCK):
                seq = idx16[c, s * SUP:(s + 1) * SUP, b].reshape(-1)
                idx_dev[c, s, b] = np.tile(
                    seq.reshape(bsz // 16, 16).T, (8, 1))
                dr_dev[c, s, b] = (
                    drel_f[c, s * SUP:(s + 1) * SUP, b]
                    .reshape(SUP * ktb, 128).T)

    dinv_pad = np.zeros(NC * SHP, np.float32)
    for c in range(NC):
        dinv_pad[c * SHP:c * SHP + SH] = dinv[c * SH:(c + 1) * SH]
    dinv_col = dinv_pad.reshape(NC, T, 128).transpose(0, 2, 1).copy()
    return idx_dev, dr_dev, dinv_col, ktb


def _build(ktb):
    nc = bacc.Bacc("TRN2", target_bir_lowering=False, debug=False,
                   enable_asserts=False, num_devices=NC)
    bsz = SUP * ktb * 128
    ap = {}

    def inp(name, shape, dt=f32):
        ap[name] = nc.dram_tensor(name, shape, dt, kind="ExternalInput").ap()
        return ap[name]

    desT = inp("desT", [768, SHP])
    tweT = inp("tweT", [768, SHP])
    numT = inp("numT", [5, SHP])
    catT = inp("catT", [3, SHP])
    idxg = inp("idxg", [NSUP, NBUCK, 128, bsz // 16], i16)
    drg = inp("drg", [NSUP, NBUCK, 128, SUP * ktb])
    dinvc = inp("dinvc", [128, T])
    iota_in = inp("iota", [128, 128])
    ident_in = inp("ident", [128, 128])
    for k, (fi, fo) in WDIMS.items():
        if fi > 128:
            inp("W_" + k, [128, (fi // 128) * fo])   # K-chunk c at cols c*fo
        else:
            inp("W_" + k, [fi, fo])
        inp("b_" + k, [fo, 1])
    outT = nc.dram_tensor("outT", [2, SHP], f32, kind="ExternalOutput").ap()

    LR = mybir.ActivationFunctionType.Lrelu
    ID = mybir.ActivationFunctionType.Identity

    with tile.TileContext(nc) as tc:
        with tc.tile_pool(name="const", bufs=1) as constp, \
             tc.tile_pool(name="state", bufs=2) as statep, \
             tc.tile_pool(name="work", bufs=4) as work, \
             tc.tile_pool(name="msg", bufs=3) as msgp, \
             tc.tile_pool(name="sel", bufs=6) as selp, \
             tc.tile_pool(name="idxp", bufs=3) as idxp, \
             tc.tile_pool(name="rhsp", bufs=3) as rhsp, \
             tc.tile_pool(name="dram", bufs=1, space="DRAM") as dram:

            W, B = {}, {}
            for k, (fi, fo) in WDIMS.items():
                if fi > 128:
                    W[k] = constp.tile([128, (fi // 128) * fo], f32, name="W" + k)
                else:
                    W[k] = constp.tile([fi, fo], f32, name="W" + k)
                nc.sync.dma_start(W[k][:], ap["W_" + k][:])
                B[k] = constp.tile([fo, 1], f32, name="B" + k)
                nc.sync.dma_start(B[k][:], ap["b_" + k][:])
            dinv_t = constp.tile([128, T], f32)
            nc.sync.dma_start(dinv_t[:], dinvc[:])
            iota_t = constp.tile([128, 128], f32)
            nc.sync.dma_start(iota_t[:], iota_in[:])
            ident_t = constp.tile([128, 128], f32)
            nc.sync.dma_start(ident_t[:], ident_in[:])

            def dense768(dst, dst0, src_hbm, wkey):
                """dst[dst0:dst0+32,:] = lrelu(W^T @ src + b), K=768 from HBM."""
                fo = 32
                with tc.tile_pool(name="psd_" + wkey, bufs=2,
                                  space="PSUM") as psp:
                    for j0 in range(0, SHP, 512):
                        n = min(512, SHP - j0)
                        ps = psp.tile([fo, 512], f32)
                        for c in range(6):
                            rt = rhsp.tile([128, 512], f32, tag="rhs768")
                            nc.sync.dma_start(
                                rt[:, :n], src_hbm[c * 128:(c + 1) * 128,
                                                   j0:j0 + n])
                            nc.tensor.matmul(
                                ps[:, :n], W[wkey][:, c * fo:(c + 1) * fo],
                                rt[:, :n], start=(c == 0), stop=(c == 5))
                        nc.scalar.activation(
                            dst[dst0:dst0 + fo, j0:j0 + n], ps[:, :n], LR,
                            bias=B[wkey][:], alpha=0.01)

            def dense_sb(dst, src, wkey, relu=True):
                """dst = act(W^T @ src + b) for SBUF-resident src, K=fi<=128."""
                fi, fo = WDIMS[wkey]
                with tc.tile_pool(name="psd_" + wkey, bufs=2,
                                  space="PSUM") as psp:
                    for j0 in range(0, SHP, 512):
                        n = min(512, SHP - j0)
                        ps = psp.tile([fo, 512], f32)
                        nc.tensor.matmul(ps[:, :n], W[wkey][:],
                                         src[:fi, j0:j0 + n],
                                         start=True, stop=True)
                        nc.scalar.activation(
                            dst[:fo, j0:j0 + n], ps[:, :n], LR if relu else ID,
                            bias=B[wkey][:], alpha=0.01 if relu else 0.0)

            # ---------------- encoder ----------------
            xT = statep.tile([128, SHP], f32, tag="x")
            dense768(xT, 0, desT, "des")
            dense768(xT, 32, tweT, "tweet")
            with tc.tile_pool(name="psnc", bufs=2, space="PSUM") as psp:
                for (w, t0, srct) in (("num", 64, numT), ("cat", 96, catT)):
                    fi, fo = WDIMS[w]
                    for j0 in range(0, SHP, 512):
                        n = min(512, SHP - j0)
                        rt = rhsp.tile([128, 512], f32, tag="rhs768")
                        nc.sync.dma_start(rt[:fi, :n], srct[:fi, j0:j0 + n])
                        ps = psp.tile([fo, 512], f32)
                        nc.tensor.matmul(ps[:, :n], W[w][:],
                                         rt[:fi, :n],
                                         start=True, stop=True)
                        nc.scalar.activation(
                            xT[t0:t0 + fo, j0:j0 + n], ps[:, :n], LR,
                            bias=B[w][:], alpha=0.01)
            x1T = statep.tile([128, SHP], f32, tag="x")
            dense_sb(x1T, xT, "inp")

            # ---------------- GCN layers ----------------
            def gcn(xin, wkey, tdt):
                hin, hout = WDIMS[wkey]
                slab = dram.tile([SHP, hout], tdt, tag="slab" + wkey)
                table = dram.tile([NP, hout], tdt, tag="tab" + wkey,
                                  addr_space="Shared")
                with tc.tile_pool(name="psh" + wkey, bufs=2,
                                  space="PSUM") as psp, \
                     tc.tile_pool(name="pst" + wkey, bufs=2,
                                  space="PSUM") as ptp:
                    for t in range(T):
                        ps = psp.tile([hout, 128], f32)
                        nc.tensor.matmul(ps[:], W[wkey][:],
                                         xin[:hin, t * 128:(t + 1) * 128],
                                         start=True, stop=True)
                        hT = work.tile([hout, 128], f32, tag="hT")
                        nc.scalar.activation(hT[:], ps[:], ID)
                        pt = ptp.tile([128, hout], f32)
                        nc.tensor.transpose(pt[:], hT[:],
                                            ident_t[:hout, :hout])
                        sl = work.tile([128, hout], tdt, tag="sl")
                        nc.scalar.activation(sl[:], pt[:], ID,
                                             scale=dinv_t[:, t:t + 1])
                        nc.sync.dma_start(slab[t * 128:(t + 1) * 128, :],
                                          sl[:])
                nc.gpsimd.collective_compute(
                    "AllGather", mybir.AluOpType.bypass,
                    replica_groups=[list(range(NC))],
                    ins=[slab[:]], outs=[table[:]])

                xo = statep.tile([hout, SHP], f32, tag="x")
                with tc.tile_pool(name="psa" + wkey, bufs=7,
                                  space="PSUM") as pag, \
                     tc.tile_pool(name="psx" + wkey, bufs=1,
                                  space="PSUM") as ptr:
                    for s in range(NSUP):
                        pss = [pag.tile([128, hout], f32, tag="agg", name=f"agg{s}_{i}")
                               for i in range(SUP)]
                        for b in range(NBUCK):
                            it = idxp.tile([128, bsz // 16], i16)
                            nc.sync.dma_start(it[:], idxg[s, b])
                            drb = idxp.tile([128, SUP * ktb], f32, tag="drb")
                            nc.sync.dma_start(drb[:], drg[s, b])
                            mt = msgp.tile([128, SUP * ktb, hout], tdt)
                            nc.gpsimd.dma_gather(
                                out_ap=mt[:], in_ap=table[b * BUCK:, :],
                                idxs_ap=it[:], num_idxs=bsz, num_idxs_reg=bsz,
                                elem_size=hout, single_packet=False)
                            for ti in range(SUP):
                                for k in range(ktb):
                                    ch = ti * ktb + k
                                    st = selp.tile([128, 128], tdt, tag="st")
                                    nc.vector.tensor_tensor(
                                        out=st[:],
                                        in0=drb[:, ch:ch + 1].to_broadcast(
                                            [128, 128]),
                                        in1=iota_t[:],
                                        op=mybir.AluOpType.is_equal)
                                    nc.tensor.matmul(
                                        pss[ti][:], st[:], mt[:, ch, :],
                                        start=(b == 0 and k == 0),
                                        stop=(b == NBUCK - 1 and
                                              k == ktb - 1))
                        for ti in range(SUP):
                            t = s * SUP + ti
                            nm = work.tile([128, hout], f32, tag="nm")
                            nc.scalar.activation(
                                nm[:], pss[ti][:], ID,
                                scale=dinv_t[:, t:t + 1])
                            pt = ptr.tile([hout, 128], f32)
                            nc.tensor.transpose(pt[:], nm[:],
                                                ident_t[:, :])
                            nc.scalar.activation(
                                xo[:hout, t * 128:(t + 1) * 128], pt[:], ID,
                                bias=B[wkey][:])
                return xo

            x2T = gcn(x1T, "g1", bf16)
            x3T = gcn(x2T, "g2", bf16)
            x4T = statep.tile([64, SHP], f32, tag="x")
            dense_sb(x4T, x3T, "o1")
            x5T = gcn(x4T, "g3", f32)
            x6T = statep.tile([64, SHP], f32, tag="x")
            dense_sb(x6T, x5T, "o2")
            x7T = statep.tile([32, SHP], f32, tag="x")
            dense_sb(x7T, x6T, "o3")
            x8T = statep.tile([16, SHP], f32, tag="x")
            dense_sb(x8T, x7T, "o4")
            x9T = statep.tile([2, SHP], f32, tag="x")
            dense_sb(x9T, x8T, "fin", relu=False)
            nc.sync.dma_start(outT[:], x9T[:2, :])

    nc.compile()
    return nc


def kernel(des, tweet, num_prop, cat_prop, edge_index, params):
    des = np.asarray(des, np.float32)
    tweet = np.asarray(tweet, np.float32)
    num_prop = np.asarray(num_prop, np.float32)
    cat_prop = np.asarray(cat_prop, np.float32)
    edge_index = np.asarray(edge_index)
    p = {k: np.asarray(v, np.float32) for k, v in params.items()}

    idx_dev, dr_dev, dinv_col, ktb = _prep(edge_index)
    nc = _build(ktb)

    iota = np.tile(np.arange(128, dtype=np.float32)[None, :], (128, 1))
    ident = np.eye(128, dtype=np.float32)

    def wmat(k):
        fi, fo = WDIMS[k]
        w = p["W_" + k]
        if fi > 128:
            return (w.reshape(fi // 128, 128, fo).transpose(1, 0, 2)
                    .reshape(128, (fi // 128) * fo).copy())
        return w.copy()

    def shardT(x, c, rows):
        out = np.zeros((rows, SHP), np.float32)
        out[:, :SH] = x[c * SH:(c + 1) * SH].T
        return out

    in_maps = []
    for c in range(NC):
        m = {"desT": shardT(des, c, 768), "tweT": shardT(tweet, c, 768),
             "numT": shardT(num_prop, c, 5), "catT": shardT(cat_prop, c, 3),
             "idxg": idx_dev[c], "drg": dr_dev[c], "dinvc": dinv_col[c],
             "iota": iota, "ident": ident}
        for k in WDIMS:
            m["W_" + k] = wmat(k)
            m["b_" + k] = p["b_" + k].reshape(-1, 1).copy()
        in_maps.append(m)

    res = run_bass_kernel_spmd(nc, in_maps, core_ids=list(range(NC)))
    out = np.zeros((N, 2), np.float32)
    for c in range(NC):
        out[c * SH:(c + 1) * SH] = res.results[c]["outT"][:, :SH].T
    return out


# revision 7
# speedup vs baseline: 1.2613x; 1.2613x over previous
"""BotGCN (nn_BotGCN_5901285065195) on 8 TRN2 NeuronCores.

Sharding: nodes split contiguously across the 8 cores (12544 padded rows per
core). Dense layers run H-major (features on partitions) so bias + LeakyReLU
fuse into single per-partition ACT ops. Each GCN layer does:
    table = AllGather(dinv * (x @ W))        # bf16 (fp32 for the H=64 layer)
    agg[d] = sum_{e: dst=d} table[src_e]     # dma_gather + selection matmuls
    x_next = dinv * agg + b
The per-edge gather uses dma_gather (int16 indices relative to one of 4
source-range buckets). The segment-sum is a matmul with a 0/1 selection
matrix built on DVE from per-edge dst values; pad edges carry dstrel=200 so
their selection column is all-zero and their gathered row is ignored.
"""
import sys

sys.path.insert(0, "/opt/trn_rl_repo")

import numpy as np
import concourse.bass as bass
import concourse.mybir as mybir
import concourse.bacc as bacc
import concourse.tile as tile
from concourse.bass_utils import run_bass_kernel_spmd

NC = 8
N = 100000
SH = 12500              # real nodes per core
SHP = 12544             # padded: 98 * 128
T = 98                  # dst tiles per core
NP = NC * SHP           # padded table rows (100352)
NBUCK = 4
BUCK = NP // NBUCK      # 25088 < 32768 (int16-safe)
SUP = 7                 # tiles per supertile; 98 = 14 * 7
NSUP = 14

f32 = mybir.dt.float32
bf16 = mybir.dt.bfloat16
i16 = mybir.dt.int16

WDIMS = {"des": (768, 32), "tweet": (768, 32), "num": (5, 32), "cat": (3, 32),
         "inp": (128, 128), "g1": (128, 128), "g2": (128, 128),
         "o1": (128, 64), "g3": (64, 64), "o2": (64, 64), "o3": (64, 32),
         "o4": (32, 16), "fin": (16, 2)}


def _prep(edge_index):
    """Host index prep -> (idx_dev, dr_dev, dinv_col, ktb)."""
    src = edge_index[0].astype(np.int64)
    dst = edge_index[1].astype(np.int64)
    loops = np.arange(N, dtype=np.int64)
    src = np.concatenate([src, loops])
    dst = np.concatenate([dst, loops])
    deg = np.bincount(dst, minlength=N).astype(np.float32)
    dinv = np.where(deg > 0, 1.0 / np.sqrt(deg), 0.0).astype(np.float32)

    gsrc = (src // SH) * SHP + (src % SH)      # padded table row of source
    core = dst // SH
    tile_id = (dst % SH) // 128
    drel = (dst % SH) % 128
    buck = gsrc % NBUCK

    order = np.lexsort((buck, tile_id, core))
    gsrc, core, tile_id, drel, buck = (a[order] for a in
                                       (gsrc, core, tile_id, drel, buck))
    key = (core * T + tile_id) * NBUCK + buck
    cnt = np.bincount(key, minlength=NC * T * NBUCK)
    ktb = int(np.ceil(cnt.max() / 128))
    slots = ktb * 128

    idx16 = np.zeros((NC * T * NBUCK, slots), np.int16)
    drel_f = np.full((NC * T * NBUCK, slots), 200.0, np.float32)
    starts = np.zeros(NC * T * NBUCK + 1, np.int64)
    np.cumsum(cnt, out=starts[1:])
    grp = np.repeat(np.arange(NC * T * NBUCK), cnt)
    pos = np.arange(len(gsrc)) - starts[grp]
    idx16[grp, pos] = (gsrc // NBUCK).astype(np.int16)
    drel_f[grp, pos] = drel.astype(np.float32)
    idx16 = idx16.reshape(NC, T, NBUCK, slots)
    drel_f = drel_f.reshape(NC, T, NBUCK, slots)

    bsz = SUP * slots
    idx_dev = np.zeros((NC, NSUP, NBUCK, 128, bsz // 16), np.int16)
    dr_dev = np.zeros((NC, NSUP, NBUCK, 128, SUP * ktb), np.float32)
    for c in range(NC):
        for s in range(NSUP):
            for b in range(NBUCK):
                seq = idx16[c, s * SUP:(s + 1) * SUP, b].reshape(-1)
                idx_dev[c, s, b] = np.tile(
                    seq.reshape(bsz // 16, 16).T, (8, 1))
                dr_dev[c, s, b] = (
                    drel_f[c, s * SUP:(s + 1) * SUP, b]
                    .reshape(SUP * ktb, 128).T)

    dinv_pad = np.zeros(NC * SHP, np.float32)
    for c in range(NC):
        dinv_pad[c * SHP:c * SHP + SH] = dinv[c * SH:(c + 1) * SH]
    dinv_col = dinv_pad.reshape(NC, T, 128).transpose(0, 2, 1).copy()
    return idx_dev, dr_dev, dinv_col, ktb


def _build(ktb):
    nc = bacc.Bacc("TRN2", target_bir_lowering=False, debug=False,
                   enable_asserts=False, num_devices=NC)
    bsz = SUP * ktb * 128
    ap = {}

    def inp(name, shape, dt=f32):
        ap[name] = nc.dram_tensor(name, shape, dt, kind="ExternalInput").ap()
        return ap[name]

    desT = inp("desT", [768, SHP])
    tweT = inp("tweT", [768, SHP])
    numT = inp("numT", [5, SHP])
    catT = inp("catT", [3, SHP])
    idxg = inp("idxg", [NSUP, NBUCK, 128, bsz // 16], i16)
    drg = inp("drg", [NSUP, NBUCK, 128, SUP * ktb])
    dinvc = inp("dinvc", [128, T])
    iota_in = inp("iota", [128, 128])
    ident_in = inp("ident", [128, 128])
    for k, (fi, fo) in WDIMS.items():
        if fi > 128:
            inp("W_" + k, [128, (fi // 128) * fo])   # K-chunk c at cols c*fo
        else:
            inp("W_" + k, [fi, fo])
        inp("b_" + k, [fo, 1])
    outT = nc.dram_tensor("outT", [2, SHP], f32, kind="ExternalOutput").ap()

    LR = mybir.ActivationFunctionType.Lrelu
    ID = mybir.ActivationFunctionType.Identity

    with tile.TileContext(nc) as tc:
        with tc.tile_pool(name="const", bufs=1) as constp, \
             tc.tile_pool(name="state", bufs=2) as statep, \
             tc.tile_pool(name="work", bufs=4) as work, \
             tc.tile_pool(name="msg", bufs=3) as msgp, \
             tc.tile_pool(name="sel", bufs=6) as selp, \
             tc.tile_pool(name="idxp", bufs=3) as idxp, \
             tc.tile_pool(name="rhsp", bufs=3) as rhsp, \
             tc.tile_pool(name="dram", bufs=1, space="DRAM") as dram:

            W, B = {}, {}
            for k, (fi, fo) in WDIMS.items():
                if fi > 128:
                    W[k] = constp.tile([128, (fi // 128) * fo], f32, name="W" + k)
                else:
                    W[k] = constp.tile([fi, fo], f32, name="W" + k)
                nc.sync.dma_start(W[k][:], ap["W_" + k][:])
                B[k] = constp.tile([fo, 1], f32, name="B" + k)
                nc.sync.dma_start(B[k][:], ap["b_" + k][:])
            dinv_t = constp.tile([128, T], f32)
            nc.sync.dma_start(dinv_t[:], dinvc[:])
            iota_t = constp.tile([128, 128], f32)
            nc.sync.dma_start(iota_t[:], iota_in[:])
            ident_t = constp.tile([128, 128], f32)
            nc.sync.dma_start(ident_t[:], ident_in[:])

            def dense768(dst, dst0, src_hbm, wkey):
                """dst[dst0:dst0+32,:] = lrelu(W^T @ src + b), K=768 from HBM."""
                fo = 32
                with tc.tile_pool(name="psd_" + wkey, bufs=2,
                                  space="PSUM") as psp:
                    for j0 in range(0, SHP, 512):
                        n = min(512, SHP - j0)
                        ps = psp.tile([fo, 512], f32)
                        for c in range(6):
                            rt = rhsp.tile([128, 512], f32, tag="rhs768")
                            nc.sync.dma_start(
                                rt[:, :n], src_hbm[c * 128:(c + 1) * 128,
                                                   j0:j0 + n])
                            nc.tensor.matmul(
                                ps[:, :n], W[wkey][:, c * fo:(c + 1) * fo],
                                rt[:, :n], start=(c == 0), stop=(c == 5))
                        nc.scalar.activation(
                            dst[dst0:dst0 + fo, j0:j0 + n], ps[:, :n], LR,
                            bias=B[wkey][:], alpha=0.01)

            def dense_sb(dst, src, wkey, relu=True):
                """dst = act(W^T @ src + b) for SBUF-resident src, K=fi<=128."""
                fi, fo = WDIMS[wkey]
                with tc.tile_pool(name="psd_" + wkey, bufs=2,
                                  space="PSUM") as psp:
                    for j0 in range(0, SHP, 512):
                        n = min(512, SHP - j0)
                        ps = psp.tile([fo, 512], f32)
                        nc.tensor.matmul(ps[:, :n], W[wkey][:],
                                         src[:fi, j0:j0 + n],
                                         start=True, stop=True)
                        nc.scalar.activation(
                            dst[:fo, j0:j0 + n], ps[:, :n], LR if relu else ID,
                            bias=B[wkey][:], alpha=0.01 if relu else 0.0)

            # ---------------- encoder ----------------
            xT = statep.tile([128, SHP], f32, tag="x")
            dense768(xT, 0, desT, "des")
            dense768(xT, 32, tweT, "tweet")
            with tc.tile_pool(name="psnc", bufs=2, space="PSUM") as psp:
                for (w, t0, srct) in (("num", 64, numT), ("cat", 96, catT)):
                    fi, fo = WDIMS[w]
                    for j0 in range(0, SHP, 512):
                        n = min(512, SHP - j0)
                        rt = rhsp.tile([128, 512], f32, tag="rhs768")
                        nc.sync.dma_start(rt[:fi, :n], srct[:fi, j0:j0 + n])
                        ps = psp.tile([fo, 512], f32)
                        nc.tensor.matmul(ps[:, :n], W[w][:],
                                         rt[:fi, :n],
                                         start=True, stop=True)
                        nc.scalar.activation(
                            xT[t0:t0 + fo, j0:j0 + n], ps[:, :n], LR,
                            bias=B[w][:], alpha=0.01)
            x1T = statep.tile([128, SHP], f32, tag="x")
            dense_sb(x1T, xT, "inp")

            # ---------------- GCN layers ----------------
            def gcn(xin, wkey, tdt):
                hin, hout = WDIMS[wkey]
                slab = dram.tile([SHP, hout], tdt, tag="slab" + wkey)
                table = dram.tile([NP, hout], tdt, tag="tab" + wkey,
                                  addr_space="Shared")
                with tc.tile_pool(name="psh" + wkey, bufs=2,
                                  space="PSUM") as psp, \
                     tc.tile_pool(name="pst" + wkey, bufs=2,
                                  space="PSUM") as ptp:
                    for t in range(T):
                        ps = psp.tile([hout, 128], f32)
                        nc.tensor.matmul(ps[:], W[wkey][:],
                                         xin[:hin, t * 128:(t + 1) * 128],
                                         start=True, stop=True)
                        hT = work.tile([hout, 128], f32, tag="hT")
                        nc.scalar.activation(hT[:], ps[:], ID)
                        pt = ptp.tile([128, hout], f32)
                        nc.tensor.transpose(pt[:], hT[:],
                                            ident_t[:hout, :hout])
                        sl = work.tile([128, hout], tdt, tag="sl")
                        nc.scalar.activation(sl[:], pt[:], ID,
                                             scale=dinv_t[:, t:t + 1])
                        nc.sync.dma_start(slab[t * 128:(t + 1) * 128, :],
                                          sl[:])
                nc.gpsimd.collective_compute(
                    "AllGather", mybir.AluOpType.bypass,
                    replica_groups=[list(range(NC))],
                    ins=[slab[:]], outs=[table[:]])

                xo = statep.tile([hout, SHP], f32, tag="x")
                with tc.tile_pool(name="psa" + wkey, bufs=7,
                                  space="PSUM") as pag, \
                     tc.tile_pool(name="psx" + wkey, bufs=1,
                                  space="PSUM") as ptr:
                    for s in range(NSUP):
                        pss = [pag.tile([128, hout], f32, tag="agg", name=f"agg{s}_{i}")
                               for i in range(SUP)]
                        for b in range(NBUCK):
                            it = idxp.tile([128, bsz // 16], i16)
                            nc.sync.dma_start(it[:], idxg[s, b])
                            drb = idxp.tile([128, SUP * ktb], f32, tag="drb")
                            nc.sync.dma_start(drb[:], drg[s, b])
                            mt = msgp.tile([128, SUP * ktb, hout], tdt)
                            tb = table[:].rearrange(
                                "(n k) h -> n (k h)", k=NBUCK)
                            nc.gpsimd.dma_gather(
                                out_ap=mt[:],
                                in_ap=tb[:, b * hout:(b + 1) * hout],
                                idxs_ap=it[:], num_idxs=bsz, num_idxs_reg=bsz,
                                elem_size=hout, elem_step=NBUCK * hout,
                                single_packet=False)
                            for ti in range(SUP):
                                for k in range(ktb):
                                    ch = ti * ktb + k
                                    st = selp.tile([128, 128], tdt, tag="st")
                                    nc.vector.tensor_tensor(
                                        out=st[:],
                                        in0=drb[:, ch:ch + 1].to_broadcast(
                                            [128, 128]),
                                        in1=iota_t[:],
                                        op=mybir.AluOpType.is_equal)
                                    nc.tensor.matmul(
                                        pss[ti][:], st[:], mt[:, ch, :],
                                        start=(b == 0 and k == 0),
                                        stop=(b == NBUCK - 1 and
                                              k == ktb - 1))
                        for ti in range(SUP):
                            t = s * SUP + ti
                            nm = work.tile([128, hout], f32, tag="nm")
                            nc.scalar.activation(
                                nm[:], pss[ti][:], ID,
                                scale=dinv_t[:, t:t + 1])
                            pt = ptr.tile([hout, 128], f32)
                            nc.tensor.transpose(pt[:], nm[:],
                                                ident_t[:, :])
                            nc.scalar.activation(
                                xo[:hout, t * 128:(t + 1) * 128], pt[:], ID,
                                bias=B[wkey][:])
                return xo

            x2T = gcn(x1T, "g1", bf16)
            x3T = gcn(x2T, "g2", bf16)
            x4T = statep.tile([64, SHP], f32, tag="x")
            dense_sb(x4T, x3T, "o1")
            x5T = gcn(x4T, "g3", f32)
            x6T = statep.tile([64, SHP], f32, tag="x")
            dense_sb(x6T, x5T, "o2")
            x7T = statep.tile([32, SHP], f32, tag="x")
            dense_sb(x7T, x6T, "o3")
            x8T = statep.tile([16, SHP], f32, tag="x")
            dense_sb(x8T, x7T, "o4")
            x9T = statep.tile([2, SHP], f32, tag="x")
            dense_sb(x9T, x8T, "fin", relu=False)
            nc.sync.dma_start(outT[:], x9T[:2, :])

    nc.compile()
    return nc


def kernel(des, tweet, num_prop, cat_prop, edge_index, params):
    des = np.asarray(des, np.float32)
    tweet = np.asarray(tweet, np.float32)
    num_prop = np.asarray(num_prop, np.float32)
    cat_prop = np.asarray(cat_prop, np.float32)
    edge_index = np.asarray(edge_index)
    p = {k: np.asarray(v, np.float32) for k, v in params.items()}

    idx_dev, dr_dev, dinv_col, ktb = _prep(edge_index)
    nc = _build(ktb)

    iota = np.tile(np.arange(128, dtype=np.float32)[None, :], (128, 1))
    ident = np.eye(128, dtype=np.float32)

    def wmat(k):
        fi, fo = WDIMS[k]
        w = p["W_" + k]
        if fi > 128:
            return (w.reshape(fi // 128, 128, fo).transpose(1, 0, 2)
                    .reshape(128, (fi // 128) * fo).copy())
        return w.copy()

    def shardT(x, c, rows):
        out = np.zeros((rows, SHP), np.float32)
        out[:, :SH] = x[c * SH:(c + 1) * SH].T
        return out

    in_maps = []
    for c in range(NC):
        m = {"desT": shardT(des, c, 768), "tweT": shardT(tweet, c, 768),
             "numT": shardT(num_prop, c, 5), "catT": shardT(cat_prop, c, 3),
             "idxg": idx_dev[c], "drg": dr_dev[c], "dinvc": dinv_col[c],
             "iota": iota, "ident": ident}
        for k in WDIMS:
            m["W_" + k] = wmat(k)
            m["b_" + k] = p["b_" + k].reshape(-1, 1).copy()
        in_maps.append(m)

    res = run_bass_kernel_spmd(nc, in_maps, core_ids=list(range(NC)))
    out = np.zeros((N, 2), np.float32)
    for c in range(NC):
        out[c * SH:(c + 1) * SH] = res.results[c]["outT"][:, :SH].T
    return out
